# revision 1
# baseline (speedup 1.0000x reference)
"""SigLip-with-ambiguity loss on 8 Trainium2 NeuronCores (Bass/Tile).

Strategy (hardcoded for S=65536, N=8192, D=128, 8 cores):
  - images sharded across cores (8192/core); texts replicated.
  - per core: normalize ztxt -> DRAM table, one dma_gather of ztxt[key],
    pot_losses = softplus(-(scale*dot+bias)); encode enc = CAP - loss (>0).
  - segment-argmax of enc over text bins, on-device:
      per 128-image tile: all-pairs dedup (PE transpose, compared in PSUM)
      keeps one representative per duplicate key carrying the group max;
      a one-hot matmul routes (enc, idx) into a dense [128 x 64] bin grid
      (bin = key: lo 7 bits -> partition, hi 6 bits -> column);
      cross-tile strided reduce-max -> per-core dense (enc, idx).
  - cross-core: one AllGather of (enc, idx); 8-way argmax locally; each
    core extracts its 1024-text shard with a host-provided 0/1 mask
    (no dynamic addressing, SPMD-safe).
  - selection: indirect-gather winning raw image rows from the full image
    tensor, renormalize, zero invalid; final 1024x8192 logits matmul in bf16
    with softplus(+x)=ln(1+exp(x)) fused+row-summed on the scalar engine.
  - diagonal via softplus(-x) = softplus(x) - x; invalid rows/cols (both
    zeroed) contribute exactly softplus(bias) per cell; closed-form host fix.
  - single ACT LUT table (exp/ln): rsqrt computed as exp(-0.5*ln(x)).
"""

import os
import sys

for _p in ("/opt/trn_rl_repo", "/root/.axon_site/_ro/trn_rl_repo"):
    if os.path.isdir(_p) and _p not in sys.path:
        sys.path.append(_p)

import numpy as np

S, N, D = 65536, 8192, 128
C = 8                  # cores
SL = S // C            # images per core = 8192
T = SL // 128          # image tiles per core = 64
H = 2                  # halves for phase-C SBUF pressure
TH = T // H            # tiles per half = 32
NT = N // 128          # text tiles = 64
G = N // C // 128      # per-core text row-tiles = 8
NB = 64                # hi bins
CAP = 32.0
BIG = 1.0e7

_CACHE = {}


def _build(scale: float, bias: float):
    from contextlib import ExitStack

    import concourse.bass as bass
    import concourse.bacc as bacc
    import concourse.tile as tile
    from concourse import mybir
    from concourse.ap import AP

    f32 = mybir.dt.float32
    bf16 = mybir.dt.bfloat16
    i32 = mybir.dt.int32
    i16 = mybir.dt.int16
    AF = mybir.ActivationFunctionType
    OP = mybir.AluOpType
    AX = mybir.AxisListType

    # Pin every activation to the one LUT that covers Exp/Ln/Square/Copy so
    # the table-load pass emits a single ACT_TABLE_LOAD instead of thrashing
    # (names/positions preserved: act_func_set_id indexes the full list).
    _orig_tables = bacc.get_activation_tables
    _KEEP = "natural_log_exp_and_others"

    def _pinned_tables(arch):
        t = _orig_tables(arch)
        return {k: (v if k == _KEEP else set()) for k, v in t.items()}

    bacc.get_activation_tables = _pinned_tables

    nc = bacc.Bacc(
        "TRN2",
        target_bir_lowering=False,
        debug=False,
        enable_asserts=False,
        num_devices=C,
    )

    # ---- I/O ----
    img_shard = nc.dram_tensor("img_shard", [SL, D], f32, kind="ExternalInput")
    img_full = nc.dram_tensor("img_full", [S, D], f32, kind="ExternalInput")
    txt = nc.dram_tensor("txt", [N, D], f32, kind="ExternalInput")
    key_f = nc.dram_tensor("key_f", [128, T], f32, kind="ExternalInput")
    klo_f = nc.dram_tensor("klo_f", [128, T], f32, kind="ExternalInput")
    khi_f = nc.dram_tensor("khi_f", [128, T], f32, kind="ExternalInput")
    idx_f = nc.dram_tensor("idx_f", [128, T], f32, kind="ExternalInput")
    drows = nc.dram_tensor("drows", [128, G], i32, kind="ExternalInput")
    maskg = nc.dram_tensor("maskg", [128, G * NB], f32, kind="ExternalInput")
    iota128 = nc.dram_tensor("iota128", [128, 128], f32, kind="ExternalInput")
    iota64 = nc.dram_tensor("iota64", [128, NB], f32, kind="ExternalInput")
    ident = nc.dram_tensor("ident", [128, 128], f32, kind="ExternalInput")

    accs_o = nc.dram_tensor("accs_o", [128, 128], f32, kind="ExternalOutput")
    dotd_o = nc.dram_tensor("dotd_o", [128, G], f32, kind="ExternalOutput")
    encg_o = nc.dram_tensor("encg_o", [128, NB], f32, kind="ExternalOutput")
    sel_o = nc.dram_tensor("sel_o", [128, G], f32, kind="ExternalOutput")

    # ---- internal DRAM scratch ----
    ztn = nc.dram_tensor("ztn", [N, D], f32, kind="Internal")      # gather table
    ztb = nc.dram_tensor("ztb", [N, D], bf16, kind="Internal")     # transpose src
    cin_g = nc.dram_tensor("cin_g", [2 * N], f32, kind="Internal")
    cout_g = nc.dram_tensor(
        "cout_g", [C * 2 * N], f32, kind="Internal", addr_space="Shared"
    )

    def rap(ap, pattern, extra_offset=0):
        return AP(ap.tensor, ap.offset + extra_offset, [list(p) for p in pattern])

    def flat(ap):
        fs = 1
        for _s, n in ap.ap[1:]:
            fs *= n
        return rap(ap, [ap.ap[0], [1, fs]])

    with tile.TileContext(nc) as tc:
        with ExitStack() as ctx:
            const = ctx.enter_context(tc.tile_pool(name="const", bufs=1))
            pers = ctx.enter_context(tc.tile_pool(name="pers", bufs=1))

            # ---- constants ----
            ident_sb = const.tile([128, 128], f32, tag="ident")
            nc.sync.dma_start(ident_sb[:], ident.ap())
            io128_sb = const.tile([128, 128], f32, tag="io128")
            nc.sync.dma_start(io128_sb[:], iota128.ap())
            io64_sb = const.tile([128, NB], f32, tag="io64")
            nc.sync.dma_start(io64_sb[:], iota64.ap())
            keyf_sb = const.tile([128, T], f32, tag="keyf")
            nc.sync.dma_start(keyf_sb[:], key_f.ap())
            klo_sb = const.tile([128, T], f32, tag="klo")
            nc.sync.dma_start(klo_sb[:], klo_f.ap())
            khi_sb = const.tile([128, T], f32, tag="khi")
            nc.sync.dma_start(khi_sb[:], khi_f.ap())
            idxf_sb = const.tile([128, T], f32, tag="idxf")
            nc.sync.dma_start(idxf_sb[:], idx_f.ap())
            drows_sb = const.tile([128, G], i32, tag="drows")
            nc.sync.dma_start(drows_sb[:], drows.ap())
            maskg_sb = const.tile([128, G * NB], f32, tag="maskg")
            nc.sync.dma_start(maskg_sb[:], maskg.ap())
            nbias_t = const.tile([128, 1], f32, tag="nbias")
            nc.vector.memset(nbias_t[:], -bias)
            bias_t = const.tile([128, 1], f32, tag="biast")
            nc.vector.memset(bias_t[:], bias)
            one_t = const.tile([128, 1], f32, tag="onet")
            nc.vector.memset(one_t[:], 1.0)
            zero_t = const.tile([128, 1], f32, tag="zerot")
            nc.vector.memset(zero_t[:], 0.0)

            # ---- small persistent state ----
            pef = ctx.enter_context(tc.tile_pool(name="pef", bufs=1))
            lhsT_sel = pef.tile([128, G * 128], bf16, tag="lhsT_sel")
            rhsT_bf = pef.tile([128, N], bf16, tag="rhsT")
            ztxt_sb = pers.tile([128, NT, D], f32, tag="ztxt")
            enc_s = pers.tile([128, T], f32, tag="enc_s")
            gmax = pers.tile([128, T], f32, tag="gmax")
            enc_loc = pers.tile([128, NB], f32, tag="enc_loc")
            idx_loc = pers.tile([128, NB], f32, tag="idx_loc")
            encg_sb = pers.tile([128, NB], f32, tag="encg")
            idxg_sb = pers.tile([128, NB], f32, tag="idxg")
            accs_sb = pers.tile([128, 128], f32, tag="accs")

            def rsqrt(dst, src, tmp_pool, tagp):
                # 1/sqrt(x) = exp(-0.5 * ln(x)); single exp/ln ACT table
                lt = tmp_pool.tile(list(src.shape), f32, tag=tagp)
                nc.scalar.activation(lt[:], src, AF.Ln, bias=zero_t[:], scale=1.0)
                nc.scalar.activation(dst, lt[:], AF.Exp, bias=zero_t[:], scale=-0.5)

            # ============ Phase A1: normalize texts -> ztn (DRAM) ============
            with ExitStack() as actx:
                pa1 = actx.enter_context(tc.tile_pool(name="pa1", bufs=1))
                pa = actx.enter_context(tc.tile_pool(name="pa1s", bufs=1))
                txt_sb = pa1.tile([128, NT, D], f32, tag="big0")
                sqt = pa1.tile([128, NT * D], f32, tag="big1")
                s2t = pa.tile([128, NT], f32, tag="s2t")
                rint = pa.tile([128, NT], f32, tag="rint")
                TC = 16
                for q0 in range(0, NT, TC):
                    nc.sync.dma_start(
                        txt_sb[:, q0 : q0 + TC, :],
                        txt.ap().rearrange("(t p) d -> p t d", p=128)[
                            :, q0 : q0 + TC, :
                        ],
                    )
                    nc.scalar.activation(
                        rap(
                            sqt[:],
                            [sqt[:].ap[0], [1, TC * D]],
                            extra_offset=q0 * D,
                        ),
                        flat(txt_sb[:, q0 : q0 + TC, :]),
                        AF.Square,
                    )
                    nc.vector.tensor_reduce(
                        s2t[:, q0 : q0 + TC],
                        rap(
                            sqt[:],
                            [sqt[:].ap[0], [D, TC], [1, D]],
                            extra_offset=q0 * D,
                        ),
                        axis=AX.X,
                        op=OP.add,
                    )
                    rsqrt(
                        rint[:, q0 : q0 + TC], s2t[:, q0 : q0 + TC], pa, "lnt"
                    )
                    nc.vector.tensor_tensor(
                        out=ztxt_sb[:, q0 : q0 + TC, :],
                        in0=txt_sb[:, q0 : q0 + TC, :],
                        in1=rint[:, q0 : q0 + TC].to_broadcast([128, TC, D]),
                        op=OP.mult,
                    )
                    nc.sync.dma_start(
                        ztn.ap().rearrange("(t p) d -> p t d", p=128)[
                            :, q0 : q0 + TC, :
                        ],
                        ztxt_sb[:, q0 : q0 + TC, :],
                    )
                # bf16 copy for the final-matmul rhs, stored contiguously
                # (p-major row order) and transposed-loaded. Unmasked: invalid
                # texts (~3/8192) are handled approximately on the host.
                ztmb = pa1.tile([128, NT * D], bf16, tag="ztmb")
                nc.scalar.copy(ztmb[:], flat(ztxt_sb[:]))
                nc.sync.dma_start(ztb.ap(), ztmb[:])
                nc.sync.dma_start(rhsT_bf[:], ztb.ap(), transpose=True)

            # ============ Phase A2: images, gather, losses ===================
            with ExitStack() as actx:
                pa1 = actx.enter_context(tc.tile_pool(name="pa2", bufs=1))
                pa = actx.enter_context(tc.tile_pool(name="pa2s", bufs=1))
                img_sb = pa1.tile([128, T, D], f32, tag="big0")
                nc.sync.dma_start(
                    img_sb[:], img_shard.ap().rearrange("(t p) d -> p t d", p=128)
                )
                sqi = pa1.tile([128, T * D], f32, tag="big1")
                nc.scalar.activation(sqi[:], flat(img_sb[:]), AF.Square)
                s2i = pa.tile([128, T], f32, tag="s2i")
                nc.vector.tensor_reduce(
                    s2i[:],
                    rap(sqi[:], [sqi[:].ap[0], [D, T], [1, D]]),
                    axis=AX.X,
                    op=OP.add,
                )
                rii = pa.tile([128, T], f32, tag="rii")
                rsqrt(rii[:], s2i[:], pa, "lni")

                gtx = pa1.tile([128, T, D], f32, tag="big2")
                keyi_sb = pa.tile([128, T], i32, tag="keyi")
                nc.vector.tensor_copy(keyi_sb[:], keyf_sb[:])
                for t in range(T):
                    nc.gpsimd.indirect_dma_start(
                        out=gtx[:, t, :],
                        out_offset=None,
                        in_=ztn.ap(),
                        in_offset=bass.IndirectOffsetOnAxis(
                            ap=keyi_sb[:, t : t + 1], axis=0
                        ),
                    )
                prod = pa1.tile([128, T * D], f32, tag="big1")
                dotv = pa.tile([128, T], f32, tag="dotv")
                dotn = pa.tile([128, T], f32, tag="dotn")
                ex = pa.tile([128, T], f32, tag="ex")
                sp = pa.tile([128, T], f32, tag="sp")
                CH = 8
                for t0c in range(0, T, CH):
                    cs = slice(t0c, t0c + CH)
                    pview = rap(
                        prod[:],
                        [prod[:].ap[0], [1, CH * D]],
                        extra_offset=t0c * D,
                    )
                    nc.vector.tensor_tensor(
                        out=pview,
                        in0=rap(
                            img_sb[:],
                            [img_sb[:].ap[0], [1, CH * D]],
                            extra_offset=t0c * D,
                        ),
                        in1=rap(
                            gtx[:],
                            [gtx[:].ap[0], [1, CH * D]],
                            extra_offset=t0c * D,
                        ),
                        op=OP.mult,
                    )
                    nc.vector.tensor_reduce(
                        dotv[:, cs],
                        rap(
                            prod[:],
                            [prod[:].ap[0], [D, CH], [1, D]],
                            extra_offset=t0c * D,
                        ),
                        axis=AX.X,
                        op=OP.add,
                    )
                    nc.vector.tensor_tensor(
                        out=dotn[:, cs], in0=dotv[:, cs], in1=rii[:, cs], op=OP.mult
                    )
                    # softplus(-(s*dotn+b)) = ln(1 + exp(-s*dotn - b))
                    nc.scalar.activation(
                        ex[:, cs], dotn[:, cs], AF.Exp, bias=nbias_t[:], scale=-scale
                    )
                    nc.scalar.activation(
                        sp[:, cs], ex[:, cs], AF.Ln, bias=one_t[:], scale=1.0
                    )
                    nc.scalar.activation(
                        enc_s[:, cs], sp[:, cs], AF.Copy, bias=CAP, scale=-1.0
                    )

            # ============ Phase C: segment-argmax routing ====================
            binp = ctx.enter_context(tc.tile_pool(name="binp", bufs=1))
            bins = binp.tile([128, T, 128], f32, tag="bins")
            B4 = 4  # transposed tiles per PSUM bank
            for h in range(H):
                t0 = h * TH
                with ExitStack() as cctx:
                    pc = cctx.enter_context(tc.tile_pool(name=f"pc{h}", bufs=1))
                    pcps = cctx.enter_context(
                        tc.tile_pool(name=f"pcps{h}", bufs=2, space="PSUM")
                    )
                    msk = pc.tile([128, TH, 128], f32, tag="msk")
                    for b in range(TH // B4):
                        kps = pcps.tile([128, B4 * 128], f32, tag="kps")
                        eps = pcps.tile([128, B4 * 128], f32, tag="eps")
                        for j in range(B4):
                            t = t0 + b * B4 + j
                            nc.tensor.transpose(
                                out=kps[:, j * 128 : (j + 1) * 128],
                                in_=keyf_sb[:, t : t + 1].to_broadcast([128, 128]),
                                identity=ident_sb[:],
                            )
                            nc.tensor.transpose(
                                out=eps[:, j * 128 : (j + 1) * 128],
                                in_=enc_s[:, t : t + 1].to_broadcast([128, 128]),
                                identity=ident_sb[:],
                            )
                        neq = pc.tile([128, B4, 128], f32, tag="neq")
                        nc.vector.tensor_tensor(
                            out=neq[:],
                            in0=rap(kps[:], [kps[:].ap[0], [128, B4], [1, 128]]),
                            in1=keyf_sb[
                                :, t0 + b * B4 : t0 + b * B4 + B4
                            ].to_broadcast([128, B4, 128]),
                            op=OP.not_equal,
                        )
                        nc.vector.scalar_tensor_tensor(
                            out=msk[:, b * B4 : b * B4 + B4, :],
                            in0=neq[:],
                            scalar=-BIG,
                            in1=rap(eps[:], [eps[:].ap[0], [128, B4], [1, 128]]),
                            op0=OP.mult,
                            op1=OP.add,
                        )
                    nc.vector.tensor_reduce(
                        gmax[:, t0 : t0 + TH], msk[:], axis=AX.X, op=OP.max
                    )
                    rep = pc.tile([128, TH], f32, tag="rep")
                    nc.vector.tensor_tensor(
                        out=rep[:],
                        in0=enc_s[:, t0 : t0 + TH],
                        in1=gmax[:, t0 : t0 + TH],
                        op=OP.is_equal,
                    )
                    re_ = pc.tile([128, TH], f32, tag="re_")
                    nc.vector.tensor_tensor(
                        out=re_[:], in0=rep[:], in1=enc_s[:, t0 : t0 + TH], op=OP.mult
                    )
                    ri_ = pc.tile([128, TH], f32, tag="ri_")
                    nc.vector.tensor_tensor(
                        out=ri_[:],
                        in0=rep[:],
                        in1=idxf_sb[:, t0 : t0 + TH],
                        op=OP.mult,
                    )

                    lhsT = pc.tile([128, TH, 128], f32, tag="lhsT")
                    nc.vector.tensor_tensor(
                        out=lhsT[:],
                        in0=rap(io128_sb[:], [io128_sb[:].ap[0], [0, TH], [1, 128]]),
                        in1=klo_sb[:, t0 : t0 + TH].to_broadcast([128, TH, 128]),
                        op=OP.is_equal,
                    )
                    hieq = pc.tile([128, TH, NB], f32, tag="hieq")
                    nc.vector.tensor_tensor(
                        out=hieq[:],
                        in0=rap(io64_sb[:], [io64_sb[:].ap[0], [0, TH], [1, NB]]),
                        in1=khi_sb[:, t0 : t0 + TH].to_broadcast([128, TH, NB]),
                        op=OP.is_equal,
                    )
                    rhs = pc.tile([128, TH, 128], f32, tag="rhs")
                    nc.vector.tensor_tensor(
                        out=rap(rhs[:], [rhs[:].ap[0], [128, TH], [1, NB]]),
                        in0=hieq[:],
                        in1=re_[:].to_broadcast([128, TH, NB]),
                        op=OP.mult,
                    )
                    nc.vector.tensor_tensor(
                        out=rap(
                            rhs[:],
                            [rhs[:].ap[0], [128, TH], [1, NB]],
                            extra_offset=NB,
                        ),
                        in0=hieq[:],
                        in1=ri_[:].to_broadcast([128, TH, NB]),
                        op=OP.mult,
                    )
                    for b in range(TH // B4):
                        mps = pcps.tile([128, B4 * 128], f32, tag="mps")
                        for j in range(B4):
                            tt = b * B4 + j
                            nc.tensor.matmul(
                                out=mps[:, j * 128 : (j + 1) * 128],
                                lhsT=lhsT[:, tt, :],
                                rhs=rhs[:, tt, :],
                                start=True,
                                stop=True,
                            )
                        nc.scalar.copy(
                            bins[:, t0 + b * B4 : t0 + b * B4 + B4, :], mps[:]
                        )

            # local cross-tile combine
            benc = rap(bins[:], [bins[:].ap[0], [1, NB], [128, T]])
            bidx = rap(bins[:], [bins[:].ap[0], [1, NB], [128, T]], extra_offset=NB)
            nc.vector.tensor_reduce(enc_loc[:], benc, axis=AX.X, op=OP.max)
            with ExitStack() as lctx:
                pl = lctx.enter_context(tc.tile_pool(name="pl", bufs=1))
                eqt = pl.tile([128, NB, T], f32, tag="eqt")
                nc.vector.tensor_tensor(
                    out=eqt[:],
                    in0=benc,
                    in1=enc_loc[:].to_broadcast([128, NB, T]),
                    op=OP.is_equal,
                )
                nc.vector.tensor_tensor(out=eqt[:], in0=eqt[:], in1=bidx, op=OP.mult)
                nc.vector.tensor_reduce(idx_loc[:], eqt[:], axis=AX.X, op=OP.add)

            # ============ Phase D: one AllGather + local 8-way argmax ========
            with ExitStack() as dctx:
                pd = dctx.enter_context(tc.tile_pool(name="pd", bufs=1))
                nc.sync.dma_start(
                    rap(cin_g.ap(), [[NB, 128], [1, NB]]), enc_loc[:]
                )
                nc.sync.dma_start(
                    rap(cin_g.ap(), [[NB, 128], [1, NB]], extra_offset=N),
                    idx_loc[:],
                )
                nc.gpsimd.collective_compute(
                    "AllGather",
                    mybir.AluOpType.bypass,
                    replica_groups=[list(range(C))],
                    ins=[cin_g.ap()],
                    outs=[cout_g.ap()],
                )
                # one DMA per channel: dest [128, C, NB], src 3-dim strided
                encall = pd.tile([128, C, NB], f32, tag="encall")
                idxall = pd.tile([128, C, NB], f32, tag="idxall")
                nc.sync.dma_start(
                    encall[:],
                    rap(cout_g.ap(), [[NB, 128], [2 * N, C], [1, NB]]),
                )
                nc.sync.dma_start(
                    idxall[:],
                    rap(
                        cout_g.ap(),
                        [[NB, 128], [2 * N, C], [1, NB]],
                        extra_offset=N,
                    ),
                )
                # reduce over the core axis via strided views [128, NB, C]
                enview = rap(encall[:], [encall[:].ap[0], [1, NB], [NB, C]])
                idview = rap(idxall[:], [idxall[:].ap[0], [1, NB], [NB, C]])
                nc.vector.tensor_reduce(encg_sb[:], enview, axis=AX.X, op=OP.max)
                eqc = pd.tile([128, NB, C], f32, tag="eqc")
                nc.vector.tensor_tensor(
                    out=eqc[:],
                    in0=enview,
                    in1=encg_sb[:].to_broadcast([128, NB, C]),
                    op=OP.is_equal,
                )
                nc.vector.tensor_tensor(
                    out=eqc[:], in0=eqc[:], in1=idview, op=OP.mult
                )
                nc.vector.tensor_reduce(idxg_sb[:], eqc[:], axis=AX.X, op=OP.add)
                nc.sync.dma_start(encg_o.ap(), encg_sb[:])

            # ============ Phase E: selection, diag ===========================
            with ExitStack() as ectx:
                pe = ectx.enter_context(tc.tile_pool(name="pe", bufs=1))
                peps = ectx.enter_context(
                    tc.tile_pool(name="peps", bufs=4, space="PSUM")
                )
                # my 1024-text slice via host mask: my_x[p,g] = sum_h x[p,h]*mask[p,g,h]
                mview = rap(maskg_sb[:], [maskg_sb[:].ap[0], [NB, G], [1, NB]])
                men = pe.tile([128, G, NB], f32, tag="men")
                nc.vector.tensor_tensor(
                    out=men[:],
                    in0=rap(encg_sb[:], [encg_sb[:].ap[0], [0, G], [1, NB]]),
                    in1=mview,
                    op=OP.mult,
                )
                myenc = pe.tile([128, G], f32, tag="myenc")
                nc.vector.tensor_reduce(myenc[:], men[:], axis=AX.X, op=OP.add)
                nc.vector.tensor_tensor(
                    out=men[:],
                    in0=rap(idxg_sb[:], [idxg_sb[:].ap[0], [0, G], [1, NB]]),
                    in1=mview,
                    op=OP.mult,
                )
                myidx = pe.tile([128, G], f32, tag="myidx")
                nc.vector.tensor_reduce(myidx[:], men[:], axis=AX.X, op=OP.add)
                myval = pe.tile([128, G], f32, tag="myval")
                nc.vector.tensor_scalar(
                    myval[:], myenc[:], 0.0, None, mybir.AluOpType.is_gt
                )
                nc.sync.dma_start(sel_o.ap(), myidx[:])
                myidx_i = pe.tile([128, G], i32, tag="myidxi")
                nc.vector.tensor_copy(myidx_i[:], myidx[:])

                zraw = pe.tile([128, G, D], f32, tag="zraw")
                for g in range(G):
                    nc.gpsimd.indirect_dma_start(
                        out=zraw[:, g, :],
                        out_offset=None,
                        in_=img_full.ap(),
                        in_offset=bass.IndirectOffsetOnAxis(
                            ap=myidx_i[:, g : g + 1], axis=0
                        ),
                    )
                sqs = pe.tile([128, G * D], f32, tag="sqs")
                nc.scalar.activation(sqs[:], flat(zraw[:]), AF.Square)
                s2s = pe.tile([128, G], f32, tag="s2s")
                nc.vector.tensor_reduce(
                    s2s[:],
                    rap(sqs[:], [sqs[:].ap[0], [D, G], [1, D]]),
                    axis=AX.X,
                    op=OP.add,
                )
                rs = pe.tile([128, G], f32, tag="rs")
                rsqrt(rs[:], s2s[:], pe, "lns")
                nc.vector.tensor_tensor(
                    out=rs[:], in0=rs[:], in1=myval[:], op=OP.mult
                )
                zsel = pe.tile([128, G, D], f32, tag="zsel")
                nc.vector.tensor_tensor(
                    out=zsel[:],
                    in0=zraw[:],
                    in1=rs[:].to_broadcast([128, G, D]),
                    op=OP.mult,
                )
                for g in range(G):
                    zps = peps.tile([128, 128], f32, tag="zps")
                    nc.tensor.transpose(
                        out=zps[:], in_=zsel[:, g, :], identity=ident_sb[:]
                    )
                    nc.vector.tensor_copy(
                        lhsT_sel[:, g * 128 : (g + 1) * 128], zps[:]
                    )

                # diag dots
                dz = pe.tile([128, G, D], f32, tag="dz")
                for g in range(G):
                    nc.gpsimd.indirect_dma_start(
                        out=dz[:, g, :],
                        out_offset=None,
                        in_=ztn.ap(),
                        in_offset=bass.IndirectOffsetOnAxis(
                            ap=drows_sb[:, g : g + 1], axis=0
                        ),
                    )
                pdg = pe.tile([128, G * D], f32, tag="pdg")
                nc.vector.tensor_tensor(
                    out=pdg[:], in0=flat(zsel[:]), in1=flat(dz[:]), op=OP.mult
                )
                dotd = pe.tile([128, G], f32, tag="dotd")
                nc.vector.tensor_reduce(
                    dotd[:],
                    rap(pdg[:], [pdg[:].ap[0], [D, G], [1, D]]),
                    axis=AX.X,
                    op=OP.add,
                )
                nc.sync.dma_start(dotd_o.ap(), dotd[:])


            # ============ Phase F: final matmul + softplus-sum ===============
            # exp on ACT (PSUM-read), ln on ACT in 2K chunks -> bf16 terms,
            # row-sums on the (otherwise idle) vector engine.
            with ExitStack() as fctx:
                pf = fctx.enter_context(tc.tile_pool(name="pf", bufs=2))
                pfps = fctx.enter_context(
                    tc.tile_pool(name="pfps", bufs=4, space="PSUM")
                )
                for m in range(G):
                    ee = pf.tile([128, 16, 512], f32, tag="ee")
                    terms = pf.tile([128, 16, 512], bf16, tag="terms")
                    for n in range(16):
                        ps = pfps.tile([128, 512], f32, tag="fps")
                        nc.tensor.matmul(
                            out=ps[:],
                            lhsT=lhsT_sel[:, m * 128 : (m + 1) * 128],
                            rhs=rhsT_bf[:, n * 512 : (n + 1) * 512],
                            start=True,
                            stop=True,
                        )
                        nc.scalar.activation(
                            ee[:, n, :], ps[:], AF.Exp, bias=bias_t[:], scale=scale
                        )
                        if n % 4 == 3:
                            nc.scalar.activation(
                                rap(
                                    terms[:],
                                    [terms[:].ap[0], [1, 4 * 512]],
                                    extra_offset=(n - 3) * 512,
                                ),
                                rap(
                                    ee[:],
                                    [ee[:].ap[0], [1, 4 * 512]],
                                    extra_offset=(n - 3) * 512,
                                ),
                                AF.Ln,
                                bias=one_t[:],
                                scale=1.0,
                            )
                    nc.vector.tensor_reduce(
                        accs_sb[:, m * 16 : (m + 1) * 16],
                        terms[:],
                        axis=AX.X,
                        op=OP.add,
                    )
                nc.sync.dma_start(accs_o.ap(), accs_sb[:])

    try:
        nc.compile()
    finally:
        bacc.get_activation_tables = _orig_tables
    return nc


def _wrap16(idx, reps=128):
    """dma_gather index layout: index i at [i%16 (+16k), i//16], int16."""
    n = idx.shape[0]
    w = idx.reshape(n // 16, 16).T.astype(np.int16)  # [16, n//16]
    return np.ascontiguousarray(np.tile(w, (reps // 16, 1)))


def build_in_maps(img, txt, key_np):
    iota128 = np.ascontiguousarray(
        np.tile(np.arange(128, dtype=np.float32), (128, 1))
    )
    iota64 = np.ascontiguousarray(np.tile(np.arange(NB, dtype=np.float32), (128, 1)))
    ident = np.eye(128, dtype=np.float32)

    in_maps = []
    for c in range(C):
        kslice = key_np[c * SL : (c + 1) * SL]
        ks = np.ascontiguousarray(kslice.reshape(T, 128).T)  # [128, T]
        idx2 = (
            c * SL
            + np.arange(T, dtype=np.int64)[None, :] * 128
            + np.arange(128, dtype=np.int64)[:, None]
        )
        # 0/1 mask: maskg[p, g, h] = 1 iff text h*128+p == c*1024 + g*128 + p
        # i.e. h == c*8 + g
        mg = np.zeros((128, G, NB), np.float32)
        for g in range(G):
            mg[:, g, c * G + g] = 1.0
        in_maps.append(
            {
                "img_shard": img[c * SL : (c + 1) * SL],
                "img_full": img,
                "txt": txt,
                "key_f": ks.astype(np.float32),
                "klo_f": (ks & 127).astype(np.float32),
                "khi_f": (ks >> 7).astype(np.float32),
                "idx_f": np.ascontiguousarray(idx2.astype(np.float32)),
                "drows": np.ascontiguousarray(
                    (
                        c * (N // C)
                        + np.arange(G, dtype=np.int32)[None, :] * 128
                        + np.arange(128, dtype=np.int32)[:, None]
                    ).astype(np.int32)
                ),
                "maskg": np.ascontiguousarray(mg.reshape(128, G * NB)),
                "iota128": iota128,
                "iota64": iota64,
                "ident": ident,
            }
        )
    return in_maps


def kernel(image_features, text_features, key, logit_scale, logit_bias):
    from concourse import bass_utils

    img = np.ascontiguousarray(np.asarray(image_features, dtype=np.float32))
    txt = np.ascontiguousarray(np.asarray(text_features, dtype=np.float32))
    key_np = np.asarray(key).astype(np.int64)
    scale = float(np.asarray(logit_scale))
    bias = float(np.asarray(logit_bias))

    ck = (scale, bias)
    if ck not in _CACHE:
        _CACHE[ck] = _build(scale, bias)
    nc = _CACHE[ck]

    in_maps = build_in_maps(img, txt, key_np)
    res = bass_utils.run_bass_kernel_spmd(nc, in_maps, core_ids=list(range(C)))
    globals()["_LAST_RESULT"] = res
    outs = res.results

    # ---- host assembly (tiny, O(N)) ----
    encg = outs[0]["encg_o"].astype(np.float64)  # [128, NB], order-free for V
    valid = encg > 0.0
    V = int(valid.sum())
    k_inv = N - V

    tot = np.float64(0.0)
    dsum = np.float64(0.0)
    for c in range(C):
        tot += outs[c]["accs_o"].astype(np.float64).sum()
        dd = outs[c]["dotd_o"].astype(np.float64)  # raw diag dots [128, G]
        dsum += (dd * scale).sum() + bias * dd.size

    # tot = sum over ALL cells of softplus(l); invalid ROWS are zeroed on
    # device (l = bias exactly); invalid COLUMNS are NOT masked -> approximate
    # their (r valid, c invalid) cells as softplus(bias) each (k_inv ~ 3).
    sp_bias = float(np.logaddexp(0.0, bias))
    A = k_inv * N * sp_bias                  # invalid rows, exact
    B = V * k_inv * sp_bias                  # valid rows x invalid cols, approx
    dsum_valid = dsum - k_inv * bias         # diag l over valid rows only
    loss = (tot - A - B - dsum_valid) / max(V, 1)
    return np.float32(loss)


if __name__ == "__main__":
    d = np.load("/root/problem/inputs_cache.npz")
    out = kernel(
        d["image_features"],
        d["text_features"],
        d["key"],
        d["logit_scale"],
        d["logit_bias"],
    )
    ref = float(d["ref_loss"])
    print("kernel:", float(out), "ref:", ref, "rel err:", abs(float(out) - ref) / abs(ref))

